# revision 7
# baseline (speedup 1.0000x reference)
"""Trainium2 Bass kernel for a 4-layer LSTM (BitcoinLSTM) + FC head.

Strategy:
  - Data-parallel over batch: B=256 -> 8 cores x 32 sequences each.
  - On each core, the 4 layers run as a wavefront over time (layer l is
    4 steps behind layer l-1), so the tensor engine always has several
    independent step-computations in flight while gate nonlinearities /
    cell updates of other layers drain.
  - Input projections are batched 4 timesteps at a time (stationary
    operand M = 4*32 = 128, full PE columns), evacuated to SBUF as fp16
    and re-injected into each step's gate PSUM with cheap K=32 identity
    matmuls.  The recurrent matmul is inherently per-step (M=32).
  - All matmul operands are bf16/fp16 with fp32 PSUM accumulation.
    Measured end-to-end output error vs the fp32 reference is ~1e-4.
  - h is produced in [batch, H] layout, cast to bf16 and transposed to
    a per-layer [H, slot, batch] ring via DMA-transpose for the next
    step's / next layer's stationary operands.
  - Biases ride the matmuls (ones-row trick); FC bias+sigmoid use the
    ACT engine's per-partition bias.

The full (unsharded) inputs come in; host-side numpy does the shard /
transpose / cast prep (free - only NEFF execution is timed), the 8
NeuronCores run SPMD, and the per-core [32,1] outputs are concatenated.
"""

import numpy as np
import ml_dtypes

import concourse.bass as bass
import concourse.mybir as mybir
import concourse.tile as tile
from concourse import bacc
from concourse.bass_utils import run_bass_kernel_spmd

BF16 = ml_dtypes.bfloat16
FP16 = np.float16

B, T, I, H, L = 256, 256, 16, 512, 4
NCORES = 8
BC = B // NCORES  # 32 sequences per core
G4 = 4 * H  # 2048
NB = G4 // 512  # 4 psum banks worth of gates
KC = H // 128  # 4 contraction chunks of 128
GP = 4  # timesteps per x-projection group
RING = 8  # h^T ring slots per layer (must be >= 2*GP)


def build_lstm_nc(t_steps: int = T):
    """Build the SPMD Bass program for one core (all cores identical)."""
    assert t_steps % GP == 0
    fdt = mybir.dt.float32
    bdt = mybir.dt.bfloat16
    hdt = mybir.dt.float16
    nc = bacc.Bacc("TRN2", target_bir_lowering=False, debug=False,
                   num_devices=NCORES)

    # ---- DRAM I/O (per-core shard, host-prepped layouts) ----
    xT_d = nc.dram_tensor("xT", [I + 1, t_steps * BC], bdt, kind="ExternalInput")
    wh_d = nc.dram_tensor("Wh", [L, KC, 128, G4], bdt, kind="ExternalInput")
    wx0_d = nc.dram_tensor("Wx0", [I + 1, G4], bdt, kind="ExternalInput")
    wxr_d = nc.dram_tensor("Wxr", [L - 1, KC, 128, G4], bdt, kind="ExternalInput")
    br_d = nc.dram_tensor("br", [1, L - 1, G4], bdt, kind="ExternalInput")
    ones_d = nc.dram_tensor("ones", [1, GP * BC], bdt, kind="ExternalInput")
    idt_d = nc.dram_tensor("idT", [128, BC], hdt, kind="ExternalInput")
    fcw_d = nc.dram_tensor("fcw", [128, KC], bdt, kind="ExternalInput")
    fcb_d = nc.dram_tensor("fcb", [BC, 1], fdt, kind="ExternalInput")
    y_d = nc.dram_tensor("y", [BC, 1], fdt, kind="ExternalOutput")

    with tile.TileContext(nc) as tc:
        with (
            tc.tile_pool(name="weights", bufs=1) as wpool,
            tc.tile_pool(name="state", bufs=1) as rpool,
            tc.tile_pool(name="cstate", bufs=2) as spool,
            tc.tile_pool(name="gates", bufs=2) as gpool,
            tc.tile_pool(name="xg", bufs=1) as xgpool,
            tc.tile_pool(name="psum", bufs=1, space="PSUM") as ppool,
        ):
            # ---- load constants to SBUF ----
            wh = wpool.tile([128, L, KC, G4], bdt)
            for l in range(L):
                for q in range(KC):
                    nc.sync.dma_start(wh[:, l, q, :], wh_d[l, q, :, :])
            wx0 = wpool.tile([I + 1, G4], bdt)
            nc.sync.dma_start(wx0[:], wx0_d[:])
            wxr = wpool.tile([128, L - 1, KC, G4], bdt)
            for l in range(L - 1):
                for q in range(KC):
                    nc.sync.dma_start(wxr[:, l, q, :], wxr_d[l, q, :, :])
            brs = wpool.tile([1, L - 1, G4], bdt)
            nc.sync.dma_start(brs[:], br_d[:])
            ones = wpool.tile([1, GP * BC], bdt)
            nc.sync.dma_start(ones[:], ones_d[:])
            idT = wpool.tile([128, BC], hdt)
            nc.sync.dma_start(idT[:], idt_d[:])
            fcw = wpool.tile([128, KC], bdt)
            nc.sync.dma_start(fcw[:], fcw_d[:])
            fcb = wpool.tile([BC, 1], fdt)
            nc.sync.dma_start(fcb[:], fcb_d[:])

            # ---- per-layer state ----
            # h^T ring: ring[p, q, s, b] = h_t[b, 128q+p] for t%RING == s
            rings = []
            for l in range(L):
                rg = rpool.tile([128, KC, RING, BC], bdt, tag=f"ring{l}",
                                name=f"ring_{l}")
                # step t=0 reads slot RING-1 as h_{-1} = 0
                nc.vector.memset(rg[:, :, RING - 1, :], 0.0)
                rings.append(rg)
            c_hist = []
            for l in range(L):
                c0 = spool.tile([BC, H], fdt, tag=f"c{l}", name=f"c0_{l}")
                nc.vector.memset(c0[:], 0.0)
                c_hist.append(c0)
            xg_sb = [None] * L  # current x-projection group per layer (fp16)
            XCH = 16  # layer-0 x chunk (timesteps per DMA)
            xt_cur = [None]

            def fetch_xchunk(t0):
                xt = xgpool.tile([I + 1, XCH * BC], bdt, tag="xt",
                                 name=f"xt_{t0}", bufs=2)
                nc.sync.dma_start(xt[:], xT_d[:, t0 * BC:(t0 + XCH) * BC])
                xt_cur[0] = xt

            sig = mybir.ActivationFunctionType.Sigmoid
            tanh = mybir.ActivationFunctionType.Tanh

            def emit_xgroup(l, t0):
                """Batched x-projection for layer l, steps t0..t0+GP-1."""
                gx = ppool.tile([GP * BC, NB, 512], fdt, tag="gx",
                                name=f"gx_{l}_{t0}")
                r0 = t0 % RING
                src = rings[l - 1]
                for q in range(KC):
                    for n in range(NB):
                        nc.tensor.matmul(
                            gx[:, n, :],
                            src[:, q, r0:r0 + GP, :],
                            wxr[:, l - 1, q, n * 512:(n + 1) * 512],
                            start=(q == 0), stop=False,
                        )
                for n in range(NB):
                    nc.tensor.matmul(
                        gx[:, n, :], ones[:], brs[:, l - 1, n * 512:(n + 1) * 512],
                        start=False, stop=True,
                    )
                xg = xgpool.tile([GP * BC, NB, 512], hdt, tag=f"xg{l}",
                                 name=f"xg_{l}_{t0}")
                nc.vector.tensor_copy(xg[:], gx[:])
                return xg

            def emit_step(l, t):
                """One recurrent step of layer l at time t."""
                g = ppool.tile([BC, NB, 512], fdt, tag="g", name=f"g_{l}_{t}")
                # x-side into psum
                if l == 0:
                    if t % XCH == 0:
                        fetch_xchunk(t)
                    tt = t % XCH
                    for n in range(NB):
                        nc.tensor.matmul(
                            g[:, n, :],
                            xt_cur[0][:, tt * BC:(tt + 1) * BC],
                            wx0[:, n * 512:(n + 1) * 512],
                            start=True, stop=False,
                        )
                else:
                    j = t % GP
                    xg = xg_sb[l]
                    for n in range(NB):
                        nc.tensor.matmul(
                            g[:, n, :],
                            idT[j * BC:(j + 1) * BC, :],
                            xg[j * BC:(j + 1) * BC, n, :],
                            start=True, stop=False,
                            tile_position=(j * BC, 0),
                        )
                # h-side (recurrent); consecutive MMs share the stationary hT
                s_prev = (t - 1) % RING
                for q in range(KC):
                    for n in range(NB):
                        nc.tensor.matmul(
                            g[:, n, :],
                            rings[l][:, q, s_prev, :],
                            wh[:, l, q, n * 512:(n + 1) * 512],
                            start=False, stop=(q == KC - 1),
                        )

                # gates: i,f,g,o in banks 0..3
                if_t = gpool.tile([BC, 2, 512], fdt, tag="if", name=f"if_{l}_{t}")
                nc.scalar.activation(if_t[:], g[:, 0:2, :], sig)
                gg_t = gpool.tile([BC, H], fdt, tag="gg", name=f"gg_{l}_{t}")
                nc.scalar.activation(gg_t[:], g[:, 2, :], tanh)
                o_t = gpool.tile([BC, H], fdt, tag="o", name=f"o_{l}_{t}")
                nc.scalar.activation(o_t[:], g[:, 3, :], sig)

                # c = f*c + i*g
                t1 = gpool.tile([BC, H], fdt, tag="t1", name=f"t1_{l}_{t}")
                nc.vector.tensor_mul(t1[:], if_t[:, 0, :], gg_t[:])
                t2 = gpool.tile([BC, H], fdt, tag="t2", name=f"t2_{l}_{t}")
                nc.vector.tensor_mul(t2[:], if_t[:, 1, :], c_hist[l][:])
                cn = spool.tile([BC, H], fdt, tag=f"c{l}", name=f"c_{l}_{t}")
                nc.vector.tensor_add(cn[:], t1[:], t2[:])
                c_hist[l] = cn

                # h = o * tanh(c), cast to bf16
                tc_t = gpool.tile([BC, H], fdt, tag="tc", name=f"tc_{l}_{t}")
                nc.scalar.activation(tc_t[:], cn[:], tanh)
                h_bf = gpool.tile([BC, H], bdt, tag="hbf", name=f"hbf_{l}_{t}")
                nc.vector.tensor_mul(h_bf[:], o_t[:], tc_t[:])

                # transpose h into the ring (both HWDGE queues)
                s = t % RING
                for q in range(KC):
                    eng = nc.sync if q % 2 == 0 else nc.scalar
                    eng.dma_start(
                        rings[l][:, q, s, :], h_bf[:, q * 128:(q + 1) * 128],
                        transpose=True,
                    )

            # ---- wavefront: layer l is 4 steps behind layer l-1 ----
            for w in range(t_steps + GP * (L - 1)):
                for l in range(L):
                    t = w - GP * l
                    if not (0 <= t < t_steps):
                        continue
                    if l > 0 and t % GP == 0:
                        xg_sb[l] = emit_xgroup(l, t)
                    emit_step(l, t)

            # ---- FC head: y = sigmoid(h_last @ fc_w.T + fc_b) ----
            gfc = ppool.tile([BC, NB, 512], fdt, tag="g", name="g_fc")
            s_last = (t_steps - 1) % RING
            for q in range(KC):
                nc.tensor.matmul(
                    gfc[:, 0, 0:1], rings[L - 1][:, q, s_last, :], fcw[:, q:q + 1],
                    start=(q == 0), stop=(q == KC - 1),
                )
            y_sb = gpool.tile([BC, 1], fdt, tag="y")
            nc.scalar.activation(y_sb[:], gfc[:, 0, 0:1], sig, bias=fcb[:])
            nc.sync.dma_start(y_d[:], y_sb[:])

    nc.compile()
    return nc


def prep_inputs(inputs, t_steps: int = T):
    """Host-side prep: shard x over cores; transpose/cast weights (shared)."""
    x = np.asarray(inputs["x"], np.float32)
    w_ih0 = np.asarray(inputs["w_ih0"], np.float32)
    w_hh0 = np.asarray(inputs["w_hh0"], np.float32)
    b_ih0 = np.asarray(inputs["b_ih0"], np.float32)
    b_hh0 = np.asarray(inputs["b_hh0"], np.float32)
    w_ih_r = np.asarray(inputs["w_ih_r"], np.float32)
    w_hh_r = np.asarray(inputs["w_hh_r"], np.float32)
    b_ih_r = np.asarray(inputs["b_ih_r"], np.float32)
    b_hh_r = np.asarray(inputs["b_hh_r"], np.float32)
    fc_w = np.asarray(inputs["fc_w"], np.float32)
    fc_b = np.asarray(inputs["fc_b"], np.float32)

    wh_all = np.concatenate([w_hh0[None], w_hh_r], 0)  # [L, 2048, 512]
    wh = np.ascontiguousarray(
        wh_all.transpose(0, 2, 1).reshape(L, KC, 128, G4)).astype(BF16)
    wx0 = np.concatenate([w_ih0.T, (b_ih0 + b_hh0)[None]], 0).astype(BF16)
    wxr = np.ascontiguousarray(
        w_ih_r.transpose(0, 2, 1).reshape(L - 1, KC, 128, G4)).astype(BF16)
    br = (b_ih_r + b_hh_r).astype(BF16)[None]
    ones = np.ones((1, GP * BC), BF16)
    idT = np.vstack([np.eye(BC, dtype=FP16)] * KC)
    fcw = np.ascontiguousarray(fc_w.reshape(KC, 128).T).astype(BF16)
    fcb = np.full((BC, 1), fc_b[0], np.float32)

    in_maps = []
    for c in range(NCORES):
        xs = x[c * BC:(c + 1) * BC, :t_steps, :]  # [BC, t, I]
        xT = np.ascontiguousarray(
            xs.transpose(2, 1, 0).reshape(I, t_steps * BC))
        xT = np.concatenate([xT, np.ones((1, t_steps * BC), np.float32)], 0)
        in_maps.append({
            "xT": xT.astype(BF16),
            "Wh": wh, "Wx0": wx0, "Wxr": wxr, "br": br,
            "ones": ones, "idT": idT, "fcw": fcw, "fcb": fcb,
        })
    return in_maps


_CACHE = {}


def _get_nc(t_steps: int = T):
    if t_steps not in _CACHE:
        _CACHE[t_steps] = build_lstm_nc(t_steps)
    return _CACHE[t_steps]


def run(inputs, t_steps: int = T, trace: bool = False):
    nc = _get_nc(t_steps)
    in_maps = prep_inputs(inputs, t_steps)
    res = run_bass_kernel_spmd(nc, in_maps, list(range(NCORES)), trace=trace)
    out = np.concatenate(
        [res.results[c]["y"] for c in range(NCORES)], 0).astype(np.float32)
    return out, res


def kernel(**inputs) -> np.ndarray:
    out, _ = run(inputs)
    return out


# revision 9
# speedup vs baseline: 1.0558x; 1.0558x over previous
"""Trainium2 Bass kernel for a 4-layer LSTM (BitcoinLSTM) + FC head.

Strategy:
  - Data-parallel over batch: B=256 -> 8 cores x 32 sequences each.
  - On each core, the 4 layers run as a wavefront over time (layer l is
    4 steps behind layer l-1), so the tensor engine always has several
    independent step-computations in flight while gate nonlinearities /
    cell updates of other layers drain.
  - Input projections are batched 4 timesteps at a time (stationary
    operand M = 4*32 = 128, full PE columns), evacuated to SBUF as fp16
    and re-injected into each step's gate PSUM with cheap K=32 identity
    matmuls.  The recurrent matmul is inherently per-step (M=32).
  - All matmul operands are bf16/fp16 with fp32 PSUM accumulation.
    Measured end-to-end output error vs the fp32 reference is ~1e-4.
  - h is produced in [batch, H] layout, cast to bf16 and transposed to
    a per-layer [H, slot, batch] ring via DMA-transpose for the next
    step's / next layer's stationary operands.
  - Biases ride the matmuls (ones-row trick); FC bias+sigmoid use the
    ACT engine's per-partition bias.

The full (unsharded) inputs come in; host-side numpy does the shard /
transpose / cast prep (free - only NEFF execution is timed), the 8
NeuronCores run SPMD, and the per-core [32,1] outputs are concatenated.
"""

import numpy as np
import ml_dtypes

import concourse.bass as bass
import concourse.mybir as mybir
import concourse.tile as tile
from concourse import bacc
from concourse.bass_utils import run_bass_kernel_spmd

BF16 = ml_dtypes.bfloat16
FP16 = np.float16

B, T, I, H, L = 256, 256, 16, 512, 4
NCORES = 8
BC = B // NCORES  # 32 sequences per core
G4 = 4 * H  # 2048
NB = G4 // 512  # 4 psum banks worth of gates
KC = H // 128  # 4 contraction chunks of 128
GP = 4  # timesteps per x-projection group
RING = 8  # h^T ring slots per layer (must be >= 2*GP)


def build_lstm_nc(t_steps: int = T):
    """Build the SPMD Bass program for one core (all cores identical)."""
    assert t_steps % GP == 0
    fdt = mybir.dt.float32
    bdt = mybir.dt.bfloat16
    hdt = mybir.dt.float16
    nc = bacc.Bacc("TRN2", target_bir_lowering=False, debug=False,
                   num_devices=NCORES)

    # ---- DRAM I/O (per-core shard, host-prepped layouts) ----
    xT_d = nc.dram_tensor("xT", [I + 1, t_steps * BC], bdt, kind="ExternalInput")
    wh_d = nc.dram_tensor("Wh", [L, KC, 128, G4], bdt, kind="ExternalInput")
    wx0_d = nc.dram_tensor("Wx0", [I + 1, G4], bdt, kind="ExternalInput")
    wxr_d = nc.dram_tensor("Wxr", [L - 1, KC, 128, G4], bdt, kind="ExternalInput")
    br_d = nc.dram_tensor("br", [1, L - 1, G4], bdt, kind="ExternalInput")
    ones_d = nc.dram_tensor("ones", [1, GP * BC], bdt, kind="ExternalInput")
    idt_d = nc.dram_tensor("idT", [128, BC], hdt, kind="ExternalInput")
    fcw_d = nc.dram_tensor("fcw", [128, KC], bdt, kind="ExternalInput")
    fcb_d = nc.dram_tensor("fcb", [BC, 1], fdt, kind="ExternalInput")
    y_d = nc.dram_tensor("y", [BC, 1], fdt, kind="ExternalOutput")

    with tile.TileContext(nc) as tc:
        with (
            tc.tile_pool(name="weights", bufs=1) as wpool,
            tc.tile_pool(name="state", bufs=1) as rpool,
            tc.tile_pool(name="cstate", bufs=2) as spool,
            tc.tile_pool(name="gates", bufs=2) as gpool,
            tc.tile_pool(name="xg", bufs=1) as xgpool,
            tc.tile_pool(name="psum", bufs=1, space="PSUM") as ppool,
        ):
            # ---- load constants to SBUF ----
            wh = wpool.tile([128, L, KC, G4], bdt)
            for l in range(L):
                for q in range(KC):
                    nc.sync.dma_start(wh[:, l, q, :], wh_d[l, q, :, :])
            wx0 = wpool.tile([I + 1, G4], bdt)
            nc.sync.dma_start(wx0[:], wx0_d[:])
            wxr = wpool.tile([128, L - 1, KC, G4], bdt)
            for l in range(L - 1):
                for q in range(KC):
                    nc.sync.dma_start(wxr[:, l, q, :], wxr_d[l, q, :, :])
            brs = wpool.tile([1, L - 1, G4], bdt)
            nc.sync.dma_start(brs[:], br_d[:])
            ones = wpool.tile([1, GP * BC], bdt)
            nc.sync.dma_start(ones[:], ones_d[:])
            idT = wpool.tile([128, BC], hdt)
            nc.sync.dma_start(idT[:], idt_d[:])
            fcw = wpool.tile([128, KC], bdt)
            nc.sync.dma_start(fcw[:], fcw_d[:])
            fcb = wpool.tile([BC, 1], fdt)
            nc.sync.dma_start(fcb[:], fcb_d[:])

            # ---- per-layer state ----
            # h^T ring: ring[p, q, s, b] = h_t[b, 128q+p] for t%RING == s
            rings = []
            for l in range(L):
                rg = rpool.tile([128, KC, RING, BC], bdt, tag=f"ring{l}",
                                name=f"ring_{l}")
                # step t=0 reads slot RING-1 as h_{-1} = 0
                nc.vector.memset(rg[:, :, RING - 1, :], 0.0)
                rings.append(rg)
            c_hist = []
            for l in range(L):
                c0 = spool.tile([BC, H], fdt, tag=f"c{l}", name=f"c0_{l}")
                nc.vector.memset(c0[:], 0.0)
                c_hist.append(c0)
            xg_sb = [None] * L  # current x-projection group per layer (fp16)
            XCH = 16  # layer-0 x chunk (timesteps per DMA)
            xt_cur = [None]

            def fetch_xchunk(t0):
                xt = xgpool.tile([I + 1, XCH * BC], bdt, tag="xt",
                                 name=f"xt_{t0}", bufs=2)
                nc.sync.dma_start(xt[:], xT_d[:, t0 * BC:(t0 + XCH) * BC])
                xt_cur[0] = xt

            sig = mybir.ActivationFunctionType.Sigmoid
            tanh = mybir.ActivationFunctionType.Tanh

            def emit_xgroup(l, t0):
                """Batched x-projection for layer l, steps t0..t0+GP-1."""
                gx = ppool.tile([GP * BC, NB, 512], fdt, tag="gx",
                                name=f"gx_{l}_{t0}")
                r0 = t0 % RING
                src = rings[l - 1]
                for q in range(KC):
                    for n in range(NB):
                        nc.tensor.matmul(
                            gx[:, n, :],
                            src[:, q, r0:r0 + GP, :],
                            wxr[:, l - 1, q, n * 512:(n + 1) * 512],
                            start=(q == 0), stop=False,
                        )
                for n in range(NB):
                    nc.tensor.matmul(
                        gx[:, n, :], ones[:], brs[:, l - 1, n * 512:(n + 1) * 512],
                        start=False, stop=True,
                    )
                xg = xgpool.tile([GP * BC, NB, 512], hdt, tag=f"xg{l}",
                                 name=f"xg_{l}_{t0}")
                for n in range(NB):
                    nc.vector.tensor_copy(xg[:, n, :], gx[:, n, :])
                return xg

            def emit_step(l, t):
                """One recurrent step of layer l at time t."""
                g = ppool.tile([BC, NB, 512], fdt, tag="g", name=f"g_{l}_{t}")
                # x-side into psum
                if l == 0:
                    if t % XCH == 0:
                        fetch_xchunk(t)
                    tt = t % XCH
                    for n in range(NB):
                        nc.tensor.matmul(
                            g[:, n, :],
                            xt_cur[0][:, tt * BC:(tt + 1) * BC],
                            wx0[:, n * 512:(n + 1) * 512],
                            start=True, stop=False,
                        )
                else:
                    j = t % GP
                    xg = xg_sb[l]
                    for n in range(NB):
                        nc.tensor.matmul(
                            g[:, n, :],
                            idT[j * BC:(j + 1) * BC, :],
                            xg[j * BC:(j + 1) * BC, n, :],
                            start=True, stop=False,
                            tile_position=(j * BC, 0),
                        )
                # h-side (recurrent); consecutive MMs share the stationary hT
                s_prev = (t - 1) % RING
                for q in range(KC):
                    for n in range(NB):
                        nc.tensor.matmul(
                            g[:, n, :],
                            rings[l][:, q, s_prev, :],
                            wh[:, l, q, n * 512:(n + 1) * 512],
                            start=False, stop=(q == KC - 1),
                        )

                # gates: i,f,g,o in banks 0..3
                if_t = gpool.tile([BC, 2, 512], fdt, tag="if", name=f"if_{l}_{t}")
                nc.scalar.activation(if_t[:], g[:, 0:2, :], sig)
                gg_t = gpool.tile([BC, H], fdt, tag="gg", name=f"gg_{l}_{t}")
                nc.scalar.activation(gg_t[:], g[:, 2, :], tanh)
                o_t = gpool.tile([BC, H], fdt, tag="o", name=f"o_{l}_{t}")
                nc.scalar.activation(o_t[:], g[:, 3, :], sig)

                # c = f*c + i*g
                t1 = gpool.tile([BC, H], fdt, tag="t1", name=f"t1_{l}_{t}")
                nc.vector.tensor_mul(t1[:], if_t[:, 0, :], gg_t[:])
                t2 = gpool.tile([BC, H], fdt, tag="t2", name=f"t2_{l}_{t}")
                nc.vector.tensor_mul(t2[:], if_t[:, 1, :], c_hist[l][:])
                cn = spool.tile([BC, H], fdt, tag=f"c{l}", name=f"c_{l}_{t}")
                nc.vector.tensor_add(cn[:], t1[:], t2[:])
                c_hist[l] = cn

                # h = o * tanh(c), cast to bf16
                tc_t = gpool.tile([BC, H], fdt, tag="tc", name=f"tc_{l}_{t}")
                nc.scalar.activation(tc_t[:], cn[:], tanh)
                h_bf = gpool.tile([BC, H], bdt, tag="hbf", name=f"hbf_{l}_{t}")
                nc.vector.tensor_mul(h_bf[:], o_t[:], tc_t[:])

                # transpose h into the ring: one DMA covers all 4 chunks
                # ([32,512] -> [128, 4, 32] with u = q*128 + p)
                s = t % RING
                nc.sync.dma_start(rings[l][:, :, s, :], h_bf[:], transpose=True)

            # ---- wavefront: layer l is SKEW steps behind layer l-1 ----
            # SKEW=5 staggers the x-group phases of layers 1..3 across waves
            SKEW = 5
            for w in range(t_steps + SKEW * (L - 1)):
                for l in range(L):
                    t = w - SKEW * l
                    if not (0 <= t < t_steps):
                        continue
                    if l > 0 and t % GP == 0:
                        xg_sb[l] = emit_xgroup(l, t)
                    emit_step(l, t)

            # ---- FC head: y = sigmoid(h_last @ fc_w.T + fc_b) ----
            gfc = ppool.tile([BC, NB, 512], fdt, tag="g", name="g_fc")
            s_last = (t_steps - 1) % RING
            for q in range(KC):
                nc.tensor.matmul(
                    gfc[:, 0, 0:1], rings[L - 1][:, q, s_last, :], fcw[:, q:q + 1],
                    start=(q == 0), stop=(q == KC - 1),
                )
            y_sb = gpool.tile([BC, 1], fdt, tag="y")
            nc.scalar.activation(y_sb[:], gfc[:, 0, 0:1], sig, bias=fcb[:])
            nc.sync.dma_start(y_d[:], y_sb[:])

    nc.compile()
    return nc


def prep_inputs(inputs, t_steps: int = T):
    """Host-side prep: shard x over cores; transpose/cast weights (shared)."""
    x = np.asarray(inputs["x"], np.float32)
    w_ih0 = np.asarray(inputs["w_ih0"], np.float32)
    w_hh0 = np.asarray(inputs["w_hh0"], np.float32)
    b_ih0 = np.asarray(inputs["b_ih0"], np.float32)
    b_hh0 = np.asarray(inputs["b_hh0"], np.float32)
    w_ih_r = np.asarray(inputs["w_ih_r"], np.float32)
    w_hh_r = np.asarray(inputs["w_hh_r"], np.float32)
    b_ih_r = np.asarray(inputs["b_ih_r"], np.float32)
    b_hh_r = np.asarray(inputs["b_hh_r"], np.float32)
    fc_w = np.asarray(inputs["fc_w"], np.float32)
    fc_b = np.asarray(inputs["fc_b"], np.float32)

    wh_all = np.concatenate([w_hh0[None], w_hh_r], 0)  # [L, 2048, 512]
    wh = np.ascontiguousarray(
        wh_all.transpose(0, 2, 1).reshape(L, KC, 128, G4)).astype(BF16)
    wx0 = np.concatenate([w_ih0.T, (b_ih0 + b_hh0)[None]], 0).astype(BF16)
    wxr = np.ascontiguousarray(
        w_ih_r.transpose(0, 2, 1).reshape(L - 1, KC, 128, G4)).astype(BF16)
    br = (b_ih_r + b_hh_r).astype(BF16)[None]
    ones = np.ones((1, GP * BC), BF16)
    idT = np.vstack([np.eye(BC, dtype=FP16)] * KC)
    fcw = np.ascontiguousarray(fc_w.reshape(KC, 128).T).astype(BF16)
    fcb = np.full((BC, 1), fc_b[0], np.float32)

    in_maps = []
    for c in range(NCORES):
        xs = x[c * BC:(c + 1) * BC, :t_steps, :]  # [BC, t, I]
        xT = np.ascontiguousarray(
            xs.transpose(2, 1, 0).reshape(I, t_steps * BC))
        xT = np.concatenate([xT, np.ones((1, t_steps * BC), np.float32)], 0)
        in_maps.append({
            "xT": xT.astype(BF16),
            "Wh": wh, "Wx0": wx0, "Wxr": wxr, "br": br,
            "ones": ones, "idT": idT, "fcw": fcw, "fcb": fcb,
        })
    return in_maps


_CACHE = {}


def _get_nc(t_steps: int = T):
    if t_steps not in _CACHE:
        _CACHE[t_steps] = build_lstm_nc(t_steps)
    return _CACHE[t_steps]


def run(inputs, t_steps: int = T, trace: bool = False):
    nc = _get_nc(t_steps)
    in_maps = prep_inputs(inputs, t_steps)
    res = run_bass_kernel_spmd(nc, in_maps, list(range(NCORES)), trace=trace)
    out = np.concatenate(
        [res.results[c]["y"] for c in range(NCORES)], 0).astype(np.float32)
    return out, res


def kernel(**inputs) -> np.ndarray:
    out, _ = run(inputs)
    return out


# revision 10
# speedup vs baseline: 1.5054x; 1.4259x over previous
"""Trainium2 Bass kernel for a 4-layer LSTM (BitcoinLSTM) + FC head.

Strategy:
  - Data-parallel over batch: B=256 -> 8 cores x 32 sequences each.
  - On each core, the 4 layers run as a wavefront over time (layer l is
    4 steps behind layer l-1), so the tensor engine always has several
    independent step-computations in flight while gate nonlinearities /
    cell updates of other layers drain.
  - Input projections are batched 4 timesteps at a time (stationary
    operand M = 4*32 = 128, full PE columns), evacuated to SBUF as fp16
    and re-injected into each step's gate PSUM with cheap K=32 identity
    matmuls.  The recurrent matmul is inherently per-step (M=32).
  - All matmul operands are bf16/fp16 with fp32 PSUM accumulation.
    Measured end-to-end output error vs the fp32 reference is ~1e-4.
  - h is produced in [batch, H] layout, cast to bf16 and transposed to
    a per-layer [H, slot, batch] ring via DMA-transpose for the next
    step's / next layer's stationary operands.
  - Biases ride the matmuls (ones-row trick); FC bias+sigmoid use the
    ACT engine's per-partition bias.

The full (unsharded) inputs come in; host-side numpy does the shard /
transpose / cast prep (free - only NEFF execution is timed), the 8
NeuronCores run SPMD, and the per-core [32,1] outputs are concatenated.
"""

import numpy as np
import ml_dtypes

import concourse.bass as bass
import concourse.mybir as mybir
import concourse.tile as tile
from concourse import bacc
from concourse.bass_utils import run_bass_kernel_spmd

BF16 = ml_dtypes.bfloat16
FP16 = np.float16

B, T, I, H, L = 256, 256, 16, 512, 4
NCORES = 8
BC = B // NCORES  # 32 sequences per core
G4 = 4 * H  # 2048
NB = G4 // 512  # 4 psum banks worth of gates
KC = H // 128  # 4 contraction chunks of 128
GP = 4  # timesteps per x-projection group
RING = 8  # h^T ring slots per layer (must be >= 2*GP)


def build_lstm_nc(t_steps: int = T):
    """Build the SPMD Bass program for one core (all cores identical)."""
    assert t_steps % GP == 0
    fdt = mybir.dt.float32
    bdt = mybir.dt.bfloat16
    hdt = mybir.dt.float16
    nc = bacc.Bacc("TRN2", target_bir_lowering=False, debug=False,
                   num_devices=NCORES)

    # ---- DRAM I/O (per-core shard, host-prepped layouts) ----
    xT_d = nc.dram_tensor("xT", [I + 1, t_steps * BC], bdt, kind="ExternalInput")
    wh_d = nc.dram_tensor("Wh", [L, KC, 128, G4], bdt, kind="ExternalInput")
    wx0_d = nc.dram_tensor("Wx0", [I + 1, G4], bdt, kind="ExternalInput")
    wxr_d = nc.dram_tensor("Wxr", [L - 1, KC, 128, G4], bdt, kind="ExternalInput")
    br_d = nc.dram_tensor("br", [1, L - 1, G4], bdt, kind="ExternalInput")
    ones_d = nc.dram_tensor("ones", [1, GP * BC], bdt, kind="ExternalInput")
    idt_d = nc.dram_tensor("idT", [128, BC], hdt, kind="ExternalInput")
    fcw_d = nc.dram_tensor("fcw", [128, KC], bdt, kind="ExternalInput")
    fcb_d = nc.dram_tensor("fcb", [BC, 1], fdt, kind="ExternalInput")
    y_d = nc.dram_tensor("y", [BC, 1], fdt, kind="ExternalOutput")

    with tile.TileContext(nc) as tc:
        with (
            tc.tile_pool(name="weights", bufs=1) as wpool,
            tc.tile_pool(name="state", bufs=1) as rpool,
            tc.tile_pool(name="cstate", bufs=2) as spool,
            tc.tile_pool(name="gates", bufs=2) as gpool,
            tc.tile_pool(name="xg", bufs=1) as xgpool,
            tc.tile_pool(name="psum", bufs=2, space="PSUM") as ppool,
        ):
            # ---- load constants to SBUF ----
            wh = wpool.tile([128, L, KC, G4], bdt)
            for l in range(L):
                for q in range(KC):
                    nc.sync.dma_start(wh[:, l, q, :], wh_d[l, q, :, :])
            wx0 = wpool.tile([I + 1, G4], bdt)
            nc.sync.dma_start(wx0[:], wx0_d[:])
            wxr = wpool.tile([128, L - 1, KC, G4], bdt)
            for l in range(L - 1):
                for q in range(KC):
                    nc.sync.dma_start(wxr[:, l, q, :], wxr_d[l, q, :, :])
            brs = wpool.tile([1, L - 1, G4], bdt)
            nc.sync.dma_start(brs[:], br_d[:])
            ones = wpool.tile([1, GP * BC], bdt)
            nc.sync.dma_start(ones[:], ones_d[:])
            idT = wpool.tile([128, BC], hdt)
            nc.sync.dma_start(idT[:], idt_d[:])
            fcw = wpool.tile([128, KC], bdt)
            nc.sync.dma_start(fcw[:], fcw_d[:])
            fcb = wpool.tile([BC, 1], fdt)
            nc.sync.dma_start(fcb[:], fcb_d[:])

            # ---- per-layer state ----
            # h^T ring: ring[p, q, s, b] = h_t[b, 128q+p] for t%RING == s
            rings = []
            for l in range(L):
                rg = rpool.tile([128, KC, RING, BC], bdt, tag=f"ring{l}",
                                name=f"ring_{l}")
                # step t=0 reads slot RING-1 as h_{-1} = 0
                nc.vector.memset(rg[:, :, RING - 1, :], 0.0)
                rings.append(rg)
            c_hist = []
            for l in range(L):
                c0 = spool.tile([BC, H], fdt, tag=f"c{l}", name=f"c0_{l}")
                nc.vector.memset(c0[:], 0.0)
                c_hist.append(c0)
            xg_sb = [None] * L  # current x-projection group per layer (fp16)
            XCH = 16  # layer-0 x chunk (timesteps per DMA)
            xt_cur = [None]

            def fetch_xchunk(t0):
                xt = xgpool.tile([I + 1, XCH * BC], bdt, tag="xt",
                                 name=f"xt_{t0}", bufs=2)
                nc.sync.dma_start(xt[:], xT_d[:, t0 * BC:(t0 + XCH) * BC])
                xt_cur[0] = xt

            sig = mybir.ActivationFunctionType.Sigmoid
            tanh = mybir.ActivationFunctionType.Tanh

            def emit_xgroup(l, t0):
                """Batched x-projection for layer l, steps t0..t0+GP-1."""
                gx = ppool.tile([GP * BC, NB, 512], fdt, tag="g",
                                name=f"gx_{l}_{t0}")
                r0 = t0 % RING
                src = rings[l - 1]
                for q in range(KC):
                    for n in range(NB):
                        nc.tensor.matmul(
                            gx[:, n, :],
                            src[:, q, r0:r0 + GP, :],
                            wxr[:, l - 1, q, n * 512:(n + 1) * 512],
                            start=(q == 0), stop=False,
                        )
                for n in range(NB):
                    nc.tensor.matmul(
                        gx[:, n, :], ones[:], brs[:, l - 1, n * 512:(n + 1) * 512],
                        start=False, stop=True,
                    )
                xg = xgpool.tile([GP * BC, NB, 512], hdt, tag=f"xg{l}",
                                 name=f"xg_{l}_{t0}")
                for n in range(NB):
                    nc.vector.tensor_copy(xg[:, n, :], gx[:, n, :])
                return xg

            def emit_step(l, t):
                """One recurrent step of layer l at time t."""
                g = ppool.tile([BC, NB, 512], fdt, tag="g", name=f"g_{l}_{t}")
                # x-side into psum
                if l == 0:
                    if t % XCH == 0:
                        fetch_xchunk(t)
                    tt = t % XCH
                    for n in range(NB):
                        nc.tensor.matmul(
                            g[:, n, :],
                            xt_cur[0][:, tt * BC:(tt + 1) * BC],
                            wx0[:, n * 512:(n + 1) * 512],
                            start=True, stop=False,
                        )
                else:
                    j = t % GP
                    xg = xg_sb[l]
                    for n in range(NB):
                        nc.tensor.matmul(
                            g[:, n, :],
                            idT[j * BC:(j + 1) * BC, :],
                            xg[j * BC:(j + 1) * BC, n, :],
                            start=True, stop=False,
                            tile_position=(j * BC, 0),
                        )
                # h-side (recurrent); consecutive MMs share the stationary hT
                s_prev = (t - 1) % RING
                for q in range(KC):
                    for n in range(NB):
                        nc.tensor.matmul(
                            g[:, n, :],
                            rings[l][:, q, s_prev, :],
                            wh[:, l, q, n * 512:(n + 1) * 512],
                            start=False, stop=(q == KC - 1),
                        )

                # gates: i,f,g,o in banks 0..3
                if_t = gpool.tile([BC, 2, 512], fdt, tag="if", name=f"if_{l}_{t}")
                nc.scalar.activation(if_t[:], g[:, 0:2, :], sig)
                gg_t = gpool.tile([BC, H], fdt, tag="gg", name=f"gg_{l}_{t}")
                nc.scalar.activation(gg_t[:], g[:, 2, :], tanh)
                o_t = gpool.tile([BC, H], fdt, tag="o", name=f"o_{l}_{t}")
                nc.scalar.activation(o_t[:], g[:, 3, :], sig)

                # c = f*c + i*g
                t1 = gpool.tile([BC, H], fdt, tag="t1", name=f"t1_{l}_{t}")
                nc.vector.tensor_mul(t1[:], if_t[:, 0, :], gg_t[:])
                t2 = gpool.tile([BC, H], fdt, tag="t2", name=f"t2_{l}_{t}")
                nc.vector.tensor_mul(t2[:], if_t[:, 1, :], c_hist[l][:])
                cn = spool.tile([BC, H], fdt, tag=f"c{l}", name=f"c_{l}_{t}")
                nc.vector.tensor_add(cn[:], t1[:], t2[:])
                c_hist[l] = cn

                # h = o * tanh(c), cast to bf16
                tc_t = gpool.tile([BC, H], fdt, tag="tc", name=f"tc_{l}_{t}")
                nc.scalar.activation(tc_t[:], cn[:], tanh)
                h_bf = gpool.tile([BC, H], bdt, tag="hbf", name=f"hbf_{l}_{t}")
                nc.vector.tensor_mul(h_bf[:], o_t[:], tc_t[:])

                # transpose h into the ring: one DMA covers all 4 chunks
                # ([32,512] -> [128, 4, 32] with u = q*128 + p)
                s = t % RING
                nc.sync.dma_start(rings[l][:, :, s, :], h_bf[:], transpose=True)

            # ---- wavefront: layer l is SKEW steps behind layer l-1 ----
            # SKEW=5 staggers the x-group phases of layers 1..3 across waves
            SKEW = 5
            for w in range(t_steps + SKEW * (L - 1)):
                for l in range(L):
                    t = w - SKEW * l
                    if not (0 <= t < t_steps):
                        continue
                    if l > 0 and t % GP == 0:
                        xg_sb[l] = emit_xgroup(l, t)
                    emit_step(l, t)

            # ---- FC head: y = sigmoid(h_last @ fc_w.T + fc_b) ----
            gfc = ppool.tile([BC, NB, 512], fdt, tag="g", name="g_fc")
            s_last = (t_steps - 1) % RING
            for q in range(KC):
                nc.tensor.matmul(
                    gfc[:, 0, 0:1], rings[L - 1][:, q, s_last, :], fcw[:, q:q + 1],
                    start=(q == 0), stop=(q == KC - 1),
                )
            y_sb = gpool.tile([BC, 1], fdt, tag="y")
            nc.scalar.activation(y_sb[:], gfc[:, 0, 0:1], sig, bias=fcb[:])
            nc.sync.dma_start(y_d[:], y_sb[:])

    nc.compile()
    return nc


def prep_inputs(inputs, t_steps: int = T):
    """Host-side prep: shard x over cores; transpose/cast weights (shared)."""
    x = np.asarray(inputs["x"], np.float32)
    w_ih0 = np.asarray(inputs["w_ih0"], np.float32)
    w_hh0 = np.asarray(inputs["w_hh0"], np.float32)
    b_ih0 = np.asarray(inputs["b_ih0"], np.float32)
    b_hh0 = np.asarray(inputs["b_hh0"], np.float32)
    w_ih_r = np.asarray(inputs["w_ih_r"], np.float32)
    w_hh_r = np.asarray(inputs["w_hh_r"], np.float32)
    b_ih_r = np.asarray(inputs["b_ih_r"], np.float32)
    b_hh_r = np.asarray(inputs["b_hh_r"], np.float32)
    fc_w = np.asarray(inputs["fc_w"], np.float32)
    fc_b = np.asarray(inputs["fc_b"], np.float32)

    wh_all = np.concatenate([w_hh0[None], w_hh_r], 0)  # [L, 2048, 512]
    wh = np.ascontiguousarray(
        wh_all.transpose(0, 2, 1).reshape(L, KC, 128, G4)).astype(BF16)
    wx0 = np.concatenate([w_ih0.T, (b_ih0 + b_hh0)[None]], 0).astype(BF16)
    wxr = np.ascontiguousarray(
        w_ih_r.transpose(0, 2, 1).reshape(L - 1, KC, 128, G4)).astype(BF16)
    br = (b_ih_r + b_hh_r).astype(BF16)[None]
    ones = np.ones((1, GP * BC), BF16)
    idT = np.vstack([np.eye(BC, dtype=FP16)] * KC)
    fcw = np.ascontiguousarray(fc_w.reshape(KC, 128).T).astype(BF16)
    fcb = np.full((BC, 1), fc_b[0], np.float32)

    in_maps = []
    for c in range(NCORES):
        xs = x[c * BC:(c + 1) * BC, :t_steps, :]  # [BC, t, I]
        xT = np.ascontiguousarray(
            xs.transpose(2, 1, 0).reshape(I, t_steps * BC))
        xT = np.concatenate([xT, np.ones((1, t_steps * BC), np.float32)], 0)
        in_maps.append({
            "xT": xT.astype(BF16),
            "Wh": wh, "Wx0": wx0, "Wxr": wxr, "br": br,
            "ones": ones, "idT": idT, "fcw": fcw, "fcb": fcb,
        })
    return in_maps


_CACHE = {}


def _get_nc(t_steps: int = T):
    if t_steps not in _CACHE:
        _CACHE[t_steps] = build_lstm_nc(t_steps)
    return _CACHE[t_steps]


def run(inputs, t_steps: int = T, trace: bool = False):
    nc = _get_nc(t_steps)
    in_maps = prep_inputs(inputs, t_steps)
    res = run_bass_kernel_spmd(nc, in_maps, list(range(NCORES)), trace=trace)
    out = np.concatenate(
        [res.results[c]["y"] for c in range(NCORES)], 0).astype(np.float32)
    return out, res


def kernel(**inputs) -> np.ndarray:
    out, _ = run(inputs)
    return out


# revision 11
# speedup vs baseline: 1.9228x; 1.2772x over previous
"""Trainium2 Bass kernel for a 4-layer LSTM (BitcoinLSTM) + FC head.

Strategy:
  - Data-parallel over batch: B=256 -> 8 cores x 32 sequences each.
  - On each core, the 4 layers run as a wavefront over time (layer l is
    4 steps behind layer l-1), so the tensor engine always has several
    independent step-computations in flight while gate nonlinearities /
    cell updates of other layers drain.
  - Input projections are batched 4 timesteps at a time (stationary
    operand M = 4*32 = 128, full PE columns), evacuated to SBUF as fp16
    and re-injected into each step's gate PSUM with cheap K=32 identity
    matmuls.  The recurrent matmul is inherently per-step (M=32).
  - All matmul operands are bf16/fp16 with fp32 PSUM accumulation.
    Measured end-to-end output error vs the fp32 reference is ~1e-4.
  - h is produced in [batch, H] layout, cast to bf16 and transposed to
    a per-layer [H, slot, batch] ring via DMA-transpose for the next
    step's / next layer's stationary operands.
  - Biases ride the matmuls (ones-row trick); FC bias+sigmoid use the
    ACT engine's per-partition bias.

The full (unsharded) inputs come in; host-side numpy does the shard /
transpose / cast prep (free - only NEFF execution is timed), the 8
NeuronCores run SPMD, and the per-core [32,1] outputs are concatenated.
"""

import numpy as np
import ml_dtypes

import concourse.bass as bass
import concourse.mybir as mybir
import concourse.tile as tile
from concourse import bacc
from concourse.bass_utils import run_bass_kernel_spmd

BF16 = ml_dtypes.bfloat16
FP16 = np.float16

B, T, I, H, L = 256, 256, 16, 512, 4
NCORES = 8
BC = B // NCORES  # 32 sequences per core
G4 = 4 * H  # 2048
NB = G4 // 512  # 4 psum banks worth of gates
KC = H // 128  # 4 contraction chunks of 128
GP = 4  # timesteps per x-projection group
RING = 8  # h^T ring slots per layer (must be >= 2*GP)


def build_lstm_nc(t_steps: int = T):
    """Build the SPMD Bass program for one core (all cores identical)."""
    assert t_steps % GP == 0
    fdt = mybir.dt.float32
    bdt = mybir.dt.bfloat16
    hdt = mybir.dt.float16
    nc = bacc.Bacc("TRN2", target_bir_lowering=False, debug=False,
                   num_devices=NCORES)

    # ---- DRAM I/O (per-core shard, host-prepped layouts) ----
    xT_d = nc.dram_tensor("xT", [I + 1, t_steps * BC], bdt, kind="ExternalInput")
    wh_d = nc.dram_tensor("Wh8", [L, 2, 128, 2, G4], mybir.dt.float8e4,
                          kind="ExternalInput")
    wx0_d = nc.dram_tensor("Wx0", [I + 1, G4], bdt, kind="ExternalInput")
    wxr_d = nc.dram_tensor("Wxr8", [L - 1, 2, 128, 2, G4], mybir.dt.float8e4,
                           kind="ExternalInput")
    br_d = nc.dram_tensor("br", [1, L - 1, G4], bdt, kind="ExternalInput")
    ones_d = nc.dram_tensor("ones", [1, GP * BC], bdt, kind="ExternalInput")
    idt_d = nc.dram_tensor("idT", [128, BC], hdt, kind="ExternalInput")
    fcw_d = nc.dram_tensor("fcw", [128, KC], bdt, kind="ExternalInput")
    fcb_d = nc.dram_tensor("fcb", [BC, 1], fdt, kind="ExternalInput")
    y_d = nc.dram_tensor("y", [BC, 1], fdt, kind="ExternalOutput")

    with tile.TileContext(nc) as tc:
        with (
            tc.tile_pool(name="weights", bufs=1) as wpool,
            tc.tile_pool(name="state", bufs=1) as rpool,
            tc.tile_pool(name="cstate", bufs=2) as spool,
            tc.tile_pool(name="gates", bufs=2) as gpool,
            tc.tile_pool(name="xg", bufs=1) as xgpool,
            tc.tile_pool(name="psum", bufs=2, space="PSUM") as ppool,
        ):
            # ---- load constants to SBUF ----
            wh8 = wpool.tile([128, L, 2, 2, G4], mybir.dt.float8e4)
            for l in range(L):
                for c in range(2):
                    nc.sync.dma_start(wh8[:, l, c, :, :], wh_d[l, c, :, :, :])
            wx0 = wpool.tile([I + 1, G4], bdt)
            nc.sync.dma_start(wx0[:], wx0_d[:])
            wxr8 = wpool.tile([128, L - 1, 2, 2, G4], mybir.dt.float8e4)
            for l in range(L - 1):
                for c in range(2):
                    nc.sync.dma_start(wxr8[:, l, c, :, :], wxr_d[l, c, :, :, :])
            brs = wpool.tile([1, L - 1, G4], bdt)
            nc.sync.dma_start(brs[:], br_d[:])
            ones = wpool.tile([1, GP * BC], bdt)
            nc.sync.dma_start(ones[:], ones_d[:])
            idT = wpool.tile([128, BC], hdt)
            nc.sync.dma_start(idT[:], idt_d[:])
            fcw = wpool.tile([128, KC], bdt)
            nc.sync.dma_start(fcw[:], fcw_d[:])
            fcb = wpool.tile([BC, 1], fdt)
            nc.sync.dma_start(fcb[:], fcb_d[:])

            # ---- per-layer state ----
            # h^T ring: ring[p, q, s, b] = h_t[b, 128q+p] for t%RING == s
            rings = []
            rings8 = []  # fp8 mirror for DoubleRow: [p, c, ko, s, b]
            for l in range(L):
                rg = rpool.tile([128, KC, RING, BC], bdt, tag=f"ring{l}",
                                name=f"ring_{l}")
                # step t=0 reads slot RING-1 as h_{-1} = 0
                nc.vector.memset(rg[:, :, RING - 1, :], 0.0)
                rings.append(rg)
                rg8 = rpool.tile([128, 2, 2, RING, BC], mybir.dt.float8e4,
                                 tag=f"ring8{l}", name=f"ring8_{l}")
                nc.vector.memset(rg8[:, :, :, RING - 1, :], 0.0)
                rings8.append(rg8)
            c_hist = []
            for l in range(L):
                c0 = spool.tile([BC, H], fdt, tag=f"c{l}", name=f"c0_{l}")
                nc.vector.memset(c0[:], 0.0)
                c_hist.append(c0)
            xg_sb = [None] * L  # current x-projection group per layer (fp16)
            XCH = 16  # layer-0 x chunk (timesteps per DMA)
            xt_cur = [None]

            def fetch_xchunk(t0):
                xt = xgpool.tile([I + 1, XCH * BC], bdt, tag="xt",
                                 name=f"xt_{t0}", bufs=2)
                nc.sync.dma_start(xt[:], xT_d[:, t0 * BC:(t0 + XCH) * BC])
                xt_cur[0] = xt

            sig = mybir.ActivationFunctionType.Sigmoid
            tanh = mybir.ActivationFunctionType.Tanh

            def emit_xgroup(l, t0):
                """Batched x-projection for layer l, steps t0..t0+GP-1."""
                gx = ppool.tile([GP * BC, NB, 512], fdt, tag="g",
                                name=f"gx_{l}_{t0}")
                r0 = t0 % RING
                src8 = rings8[l - 1]
                for c in range(2):
                    for n in range(NB):
                        nc.tensor.matmul(
                            gx[:, n, :],
                            src8[:, c, :, r0:r0 + GP, :],
                            wxr8[:, l - 1, c, :, n * 512:(n + 1) * 512],
                            start=(c == 0), stop=False,
                            perf_mode=mybir.MatmulPerfMode.DoubleRow,
                        )
                for n in range(NB):
                    nc.tensor.matmul(
                        gx[:, n, :], ones[:], brs[:, l - 1, n * 512:(n + 1) * 512],
                        start=False, stop=True,
                    )
                xg = xgpool.tile([GP * BC, NB, 512], hdt, tag=f"xg{l}",
                                 name=f"xg_{l}_{t0}")
                for n in range(NB):
                    nc.vector.tensor_copy(xg[:, n, :], gx[:, n, :])
                return xg

            def emit_step(l, t):
                """One recurrent step of layer l at time t."""
                g = ppool.tile([BC, NB, 512], fdt, tag="g", name=f"g_{l}_{t}")
                # x-side into psum
                if l == 0:
                    if t % XCH == 0:
                        fetch_xchunk(t)
                    tt = t % XCH
                    for n in range(NB):
                        nc.tensor.matmul(
                            g[:, n, :],
                            xt_cur[0][:, tt * BC:(tt + 1) * BC],
                            wx0[:, n * 512:(n + 1) * 512],
                            start=True, stop=False,
                        )
                else:
                    j = t % GP
                    xg = xg_sb[l]
                    for n in range(NB):
                        nc.tensor.matmul(
                            g[:, n, :],
                            idT[j * BC:(j + 1) * BC, :],
                            xg[j * BC:(j + 1) * BC, n, :],
                            start=True, stop=False,
                            tile_position=(j * BC, 0),
                        )
                # h-side (recurrent), fp8 DoubleRow: 2 MMs contract K=256 each
                s_prev = (t - 1) % RING
                for c in range(2):
                    for n in range(NB):
                        nc.tensor.matmul(
                            g[:, n, :],
                            rings8[l][:, c, :, s_prev, :],
                            wh8[:, l, c, :, n * 512:(n + 1) * 512],
                            start=False, stop=(c == 1),
                            perf_mode=mybir.MatmulPerfMode.DoubleRow,
                        )

                # gates in permuted order i,f,o,g (banks 0..3)
                ifo_t = gpool.tile([BC, 3, 512], fdt, tag="ifo", name=f"ifo_{l}_{t}")
                nc.scalar.activation(ifo_t[:], g[:, 0:3, :], sig)
                gg_t = gpool.tile([BC, H], fdt, tag="gg", name=f"gg_{l}_{t}")
                nc.scalar.activation(gg_t[:], g[:, 3, :], tanh)

                # c = f*c + i*g
                t1 = gpool.tile([BC, H], fdt, tag="t1", name=f"t1_{l}_{t}")
                nc.vector.tensor_mul(t1[:], ifo_t[:, 0, :], gg_t[:])
                t2 = gpool.tile([BC, H], fdt, tag="t2", name=f"t2_{l}_{t}")
                nc.vector.tensor_mul(t2[:], ifo_t[:, 1, :], c_hist[l][:])
                cn = spool.tile([BC, H], fdt, tag=f"c{l}", name=f"c_{l}_{t}")
                nc.vector.tensor_add(cn[:], t1[:], t2[:])
                c_hist[l] = cn

                # h = o * tanh(c), cast to bf16
                tc_t = gpool.tile([BC, H], fdt, tag="tc", name=f"tc_{l}_{t}")
                nc.scalar.activation(tc_t[:], cn[:], tanh)
                h_bf = gpool.tile([BC, H], bdt, tag="hbf", name=f"hbf_{l}_{t}")
                nc.vector.tensor_mul(h_bf[:], ifo_t[:, 2, :], tc_t[:])

                # transpose h into the ring: one DMA covers all 4 chunks
                # ([32,512] -> [128, 4, 32] with u = q*128 + p)
                s = t % RING
                nc.sync.dma_start(rings[l][:, :, s, :], h_bf[:], transpose=True)
                # fp8 mirror for the DoubleRow matmuls
                nc.vector.tensor_copy(
                    rings8[l][:, :, :, s, :],
                    rings[l][:, :, s, :].rearrange("p (c k) b -> p c k b", c=2),
                )

            # ---- wavefront: layer l is SKEW steps behind layer l-1 ----
            # SKEW=5 staggers the x-group phases of layers 1..3 across waves
            SKEW = 5
            for w in range(t_steps + SKEW * (L - 1)):
                for l in range(L):
                    t = w - SKEW * l
                    if not (0 <= t < t_steps):
                        continue
                    if l > 0 and t % GP == 0:
                        xg_sb[l] = emit_xgroup(l, t)
                    emit_step(l, t)

            # ---- FC head: y = sigmoid(h_last @ fc_w.T + fc_b) ----
            gfc = ppool.tile([BC, NB, 512], fdt, tag="g", name="g_fc")
            s_last = (t_steps - 1) % RING
            for q in range(KC):
                nc.tensor.matmul(
                    gfc[:, 0, 0:1], rings[L - 1][:, q, s_last, :], fcw[:, q:q + 1],
                    start=(q == 0), stop=(q == KC - 1),
                )
            y_sb = gpool.tile([BC, 1], fdt, tag="y")
            nc.scalar.activation(y_sb[:], gfc[:, 0, 0:1], sig, bias=fcb[:])
            nc.sync.dma_start(y_d[:], y_sb[:])

    nc.compile()
    return nc


def prep_inputs(inputs, t_steps: int = T):
    """Host-side prep: shard x over cores; transpose/cast weights (shared)."""
    x = np.asarray(inputs["x"], np.float32)
    w_ih0 = np.asarray(inputs["w_ih0"], np.float32)
    w_hh0 = np.asarray(inputs["w_hh0"], np.float32)
    b_ih0 = np.asarray(inputs["b_ih0"], np.float32)
    b_hh0 = np.asarray(inputs["b_hh0"], np.float32)
    w_ih_r = np.asarray(inputs["w_ih_r"], np.float32)
    w_hh_r = np.asarray(inputs["w_hh_r"], np.float32)
    b_ih_r = np.asarray(inputs["b_ih_r"], np.float32)
    b_hh_r = np.asarray(inputs["b_hh_r"], np.float32)
    fc_w = np.asarray(inputs["fc_w"], np.float32)
    fc_b = np.asarray(inputs["fc_b"], np.float32)

    FP8 = ml_dtypes.float8_e4m3
    # permute gate blocks from torch order (i,f,g,o) to (i,f,o,g) so one
    # sigmoid covers banks 0..2
    PERM = [0, 1, 3, 2]

    def perm_g(w):  # permute along the 4H axis (axis -2 of [..., 4H, K])
        shp = w.shape
        return w.reshape(shp[:-2] + (4, H) + shp[-1:])[..., PERM, :, :].reshape(shp)

    def perm_b(b):  # [..., 4H]
        shp = b.shape
        return b.reshape(shp[:-1] + (4, H))[..., PERM, :].reshape(shp)

    w_hh0 = perm_g(w_hh0[None])[0]
    w_hh_r = perm_g(w_hh_r)
    w_ih0 = perm_g(w_ih0[None])[0]
    w_ih_r = perm_g(w_ih_r)
    b0 = perm_b(b_ih0 + b_hh0)
    br_v = perm_b(b_ih_r + b_hh_r)

    wh_all = np.concatenate([w_hh0[None], w_hh_r], 0)  # [L, 2048, 512]
    # DoubleRow fp8 layout: [L, c, ki, ko, n] with u = 256c + 128ko + ki
    wh8 = np.ascontiguousarray(
        wh_all.transpose(0, 2, 1).reshape(L, 2, 2, 128, G4).transpose(0, 1, 3, 2, 4)
    ).astype(FP8)
    wx0 = np.concatenate([w_ih0.T, b0[None]], 0).astype(BF16)
    wxr8 = np.ascontiguousarray(
        w_ih_r.transpose(0, 2, 1).reshape(L - 1, 2, 2, 128, G4).transpose(0, 1, 3, 2, 4)
    ).astype(FP8)
    br = br_v.astype(BF16)[None]
    ones = np.ones((1, GP * BC), BF16)
    idT = np.vstack([np.eye(BC, dtype=FP16)] * KC)
    fcw = np.ascontiguousarray(fc_w.reshape(KC, 128).T).astype(BF16)
    fcb = np.full((BC, 1), fc_b[0], np.float32)

    in_maps = []
    for c in range(NCORES):
        xs = x[c * BC:(c + 1) * BC, :t_steps, :]  # [BC, t, I]
        xT = np.ascontiguousarray(
            xs.transpose(2, 1, 0).reshape(I, t_steps * BC))
        xT = np.concatenate([xT, np.ones((1, t_steps * BC), np.float32)], 0)
        in_maps.append({
            "xT": xT.astype(BF16),
            "Wh8": wh8, "Wx0": wx0, "Wxr8": wxr8, "br": br,
            "ones": ones, "idT": idT, "fcw": fcw, "fcb": fcb,
        })
    return in_maps


_CACHE = {}


def _get_nc(t_steps: int = T):
    if t_steps not in _CACHE:
        _CACHE[t_steps] = build_lstm_nc(t_steps)
    return _CACHE[t_steps]


def run(inputs, t_steps: int = T, trace: bool = False):
    nc = _get_nc(t_steps)
    in_maps = prep_inputs(inputs, t_steps)
    res = run_bass_kernel_spmd(nc, in_maps, list(range(NCORES)), trace=trace)
    out = np.concatenate(
        [res.results[c]["y"] for c in range(NCORES)], 0).astype(np.float32)
    return out, res


def kernel(**inputs) -> np.ndarray:
    out, _ = run(inputs)
    return out
